# revision 9
# baseline (speedup 1.0000x reference)
"""Trainium2 Bass kernel for nn_Decoder_14680198217759.

Multi-head attention decoder (B=32, G=N=512, E=128, H=8, D=16), pure data
parallel over 8 NeuronCores (4 batches/core), fp32 throughout.

Layout strategy per batch (all on one core):
  - Activations transposed via PE so E sits on partitions: xT [E, G].
  - Projections produce qT/kT in a head-padded layout (head j of a 4-head
    group at partition offset 32j) so per-head K=16 score matmuls can use
    legal base partitions and 2-way PE row tiling.
  - Scores computed TRANSPOSED: scoreT_h [n, g] (head pair packed into one
    [128, 1024] PSUM tile), so the softmax(n) contraction of PV needs no
    transpose of the big prob matrix.
  - The rank-3 mask rides the PE: maskT is added into the score PSUM via an
    identity-matmul (lhsT = natural mask chunk, rhs = I) per head.
  - exp on ACT (PSUM -> SBUF). PV contracts exp with v_aug (v plus a ones
    column) giving out^T and the softmax denominators in one pass.
  - Per-head normalization happens in natural space: PV output transposed
    back [g, hd], reciprocal on the strided ones columns, per-partition
    tensor_scalar multiplies, transpose again for the combine matmul.
  - Branch 2 (single-head scoring) stays natural [g, n]: tanh on ACT with
    input scale 1/sqrt(E); mask added with one DVE tensor_tensor (the mask is
    scale invariant: 10*(t + m) masks as well as 10 t + m); exp with scale=10
    and accum_out producing the denominators for free.
"""

import numpy as np

B, G, N, E, H, D = 32, 512, 512, 128, 8, 16
SQRT_E = 11.313708498984761
NCORES = 8
BL = B // NCORES  # batches per core

_CACHE = {}


# --------------------------------------------------------------------------
# BIR wait legalization: this toolchain's walrus accepts at most ONE sem wait
# per instruction; Tile's scheduler can emit more (notably on the kernel-tail
# drain). Split excess waits onto same-engine NoOps placed directly before
# the offending instruction (same-queue program order keeps the semantics).
# --------------------------------------------------------------------------
def _legalize_waits(nc, max_waits=1):
    import concourse.mybir as mybir

    n_split = 0
    for f in nc.m.functions:
        for bb in f.blocks:
            out = []
            for ins in bb.instructions:
                si = ins.sync_info
                waits = list(si.on_wait) if si and si.on_wait else []
                if len(waits) > max_waits:
                    while len(waits) > max_waits:
                        chunk, waits = waits[:max_waits], waits[max_waits:]
                        nop = mybir.InstNoOp(
                            name=f"I-waitfix-{nc.next_id()}", ins=[], outs=[]
                        )
                        nop.engine = ins.engine
                        nop.sync_info = mybir.SyncInfo(on_wait=chunk, on_update=[])
                        out.append(nop)
                        n_split += 1
                    ins.sync_info = mybir.SyncInfo(
                        on_wait=waits, on_update=list(si.on_update or [])
                    )
                out.append(ins)
            bb.instructions[:] = out
    return n_split


def _build_nc(legalize=True):
    import concourse.bass as bass
    import concourse.mybir as mybir
    import concourse.tile as tile
    from concourse.masks import make_identity

    f32 = mybir.dt.float32
    AF = mybir.ActivationFunctionType

    nc = bass.Bass()

    nodes_d = nc.dram_tensor("nodes", [BL, N, E], f32, kind="ExternalInput")
    q1_d = nc.dram_tensor("q1", [BL, G, E], f32, kind="ExternalInput")
    last_d = nc.dram_tensor("last", [BL, G, E], f32, kind="ExternalInput")
    mask_d = nc.dram_tensor("mask", [BL, G, N], f32, kind="ExternalInput")
    wnames = ["Wq1p0", "Wq1p1", "Wqlp0", "Wqlp1", "Wkp0", "Wkp1", "Wv", "Wc"]
    w_d = {n: nc.dram_tensor(n, [E, 128], f32, kind="ExternalInput") for n in wnames}
    b_d = nc.dram_tensor("bc", [E, 1], f32, kind="ExternalInput")
    probs_d = nc.dram_tensor("probs", [BL, G, N], f32, kind="ExternalOutput")

    with tile.TileContext(nc) as tc:
        import contextlib

        with contextlib.ExitStack() as ctx:
            pw = ctx.enter_context(tc.tile_pool(name="pw", bufs=1))
            pin = ctx.enter_context(tc.tile_pool(name="pin", bufs=2))
            pxt = ctx.enter_context(tc.tile_pool(name="pxt", bufs=2))
            pproj = ctx.enter_context(tc.tile_pool(name="pproj", bufs=2))
            pexp = ctx.enter_context(tc.tile_pool(name="pexp", bufs=6))
            pmisc = ctx.enter_context(tc.tile_pool(name="pmisc", bufs=2))
            pstage = ctx.enter_context(tc.tile_pool(name="pstage", bufs=2))
            ps_score = ctx.enter_context(
                tc.tile_pool(name="ps_score", bufs=2, space="PSUM")
            )
            ps_pv = ctx.enter_context(tc.tile_pool(name="ps_pv", bufs=2, space="PSUM"))
            ps_m = ctx.enter_context(tc.tile_pool(name="ps_m", bufs=2, space="PSUM"))

            # ---- constants / weights (once) ----
            ident = pw.tile([128, 128], f32)
            make_identity(nc, ident)
            w_sb = {}
            for n in wnames:
                w_sb[n] = pw.tile([128, 128], f32, name=f"w_{n}", tag=f"w_{n}")
                nc.sync.dma_start(out=w_sb[n], in_=w_d[n][:, :])
            b_sb = pw.tile([128, 1], f32)
            nc.sync.dma_start(out=b_sb, in_=b_d[:, :])
            # v_aug: per n-chunk, 8 heads at 32-col blocks: cols 32h..32h+15 =
            # v head h, col 32h+16 = 1.0 (denominator row), rest zero.
            v_aug = pw.tile([128, 4, 256], f32)
            nc.vector.memset(v_aug, 0.0)
            v_aug_blk = v_aug.rearrange("p c (h x) -> p c h x", x=32)
            nc.vector.memset(v_aug_blk[:, :, :, 16:17], 1.0)

            for b in range(BL):
                # ---- loads ----
                x_nodes = pin.tile([128, 4, 128], f32)
                nc.sync.dma_start(
                    out=x_nodes, in_=nodes_d[b].rearrange("(c p) e -> p c e", p=128)
                )
                x_q1 = pin.tile([128, 4, 128], f32)
                nc.sync.dma_start(
                    out=x_q1, in_=q1_d[b].rearrange("(c p) e -> p c e", p=128)
                )
                x_last = pin.tile([128, 4, 128], f32)
                nc.sync.dma_start(
                    out=x_last, in_=last_d[b].rearrange("(c p) e -> p c e", p=128)
                )
                mask_t = pin.tile([128, 4, 512], f32)
                nc.sync.dma_start(
                    out=mask_t, in_=mask_d[b].rearrange("(c p) n -> p c n", p=128)
                )

                # ---- transpose activations to [E, G] ----
                def transpose_to(dst, src_nat):
                    for c in range(4):
                        tp = ps_m.tile([128, 128], f32, tag="m")
                        nc.tensor.transpose(tp, src_nat[:, c, :], ident)
                        nc.vector.tensor_copy(dst[:, c * 128 : (c + 1) * 128], tp)

                nodesT = pxt.tile([128, 512], f32)
                transpose_to(nodesT, x_nodes)
                q1T = pxt.tile([128, 512], f32)
                transpose_to(q1T, x_q1)
                lastT = pxt.tile([128, 512], f32)
                transpose_to(lastT, x_last)

                # ---- projections (outputs transposed/padded) ----
                qT = []
                for g4 in range(2):  # head groups 0-3 / 4-7
                    ps = ps_m.tile([128, 512], f32, tag="m")
                    nc.tensor.matmul(
                        ps, w_sb[f"Wq1p{g4}"], q1T, start=True, stop=False
                    )
                    nc.tensor.matmul(
                        ps, w_sb[f"Wqlp{g4}"], lastT, start=False, stop=True
                    )
                    t = pproj.tile([128, 512], f32, tag=f"qT{g4}")
                    nc.vector.tensor_copy(t, ps)
                    qT.append(t)
                kT = []
                for g4 in range(2):
                    ps = ps_m.tile([128, 512], f32, tag="m")
                    nc.tensor.matmul(ps, w_sb[f"Wkp{g4}"], nodesT, start=True, stop=True)
                    t = pproj.tile([128, 512], f32, tag=f"kT{g4}")
                    nc.vector.tensor_copy(t, ps)
                    kT.append(t)
                # v natural [n, hd] scattered into v_aug 32-blocks
                for c in range(4):
                    ps = ps_m.tile([128, 128], f32, tag="m")
                    nc.tensor.matmul(
                        ps,
                        nodesT[:, c * 128 : (c + 1) * 128],
                        w_sb["Wv"],
                        start=True,
                        stop=True,
                    )
                    nc.vector.tensor_copy(
                        v_aug_blk[:, c, :, 0:16],
                        ps.rearrange("p (h d) -> p h d", d=16),
                    )

                # ---- attention: scores (transposed) + exp + PV ----
                pv_banks = []
                for half in range(2):  # heads 0-3 then 4-7
                    pv = ps_pv.tile([128, 512], f32, tag="pv")
                    pv_banks.append(pv)
                    for hp in range(2):  # head pair within group
                        j0, j1 = 2 * hp, 2 * hp + 1  # pad-slot indices
                        expps = []
                        for c in range(4):  # n-chunks
                            sc = ps_score.tile([128, 1024], f32, tag="sc")
                            for idx, j in enumerate((j0, j1)):
                                nc.tensor.matmul(
                                    sc[:, idx * 512 : (idx + 1) * 512],
                                    kT[half][
                                        32 * j : 32 * j + 16,
                                        c * 128 : (c + 1) * 128,
                                    ],
                                    qT[half][32 * j : 32 * j + 16, :],
                                    start=True,
                                    stop=False,
                                    tile_position=(32 * j, 0),
                                )
                            # += maskT via identity trick, per head
                            for idx in range(2):
                                for gc in range(4):
                                    nc.tensor.matmul(
                                        sc[
                                            :,
                                            idx * 512
                                            + gc * 128 : idx * 512
                                            + (gc + 1) * 128,
                                        ],
                                        mask_t[:, gc, c * 128 : (c + 1) * 128],
                                        ident,
                                        start=False,
                                        stop=(gc == 3),
                                    )
                            expp = pexp.tile([128, 1024], f32, tag="expp")
                            nc.scalar.activation(expp, sc, AF.Exp)
                            expps.append(expp)
                        # PV accumulate: out_aug^T rows 32j. The psum group
                        # checker requires groups in one bank to be strictly
                        # sequential, so each head's 4 n-chunk MMs run
                        # back-to-back as one start..stop group.
                        for idx, j in enumerate((j0, j1)):
                            vcol = 32 * (half * 4 + j)  # head's v_aug block
                            for c in range(4):
                                nc.tensor.matmul(
                                    pv[32 * j : 32 * j + 32, :],
                                    v_aug[:, c, vcol : vcol + 32],
                                    expps[c][:, idx * 512 : (idx + 1) * 512],
                                    start=(c == 0),
                                    stop=(c == 3),
                                    tile_position=(0, 32 * j),
                                )

                # ---- normalize per head, in natural space ----
                u_nat = pmisc.tile([128, 4, 256], f32)
                for half in range(2):
                    uT_pad = pmisc.tile([128, 512], f32, tag="uT_pad")
                    nc.vector.tensor_copy(uT_pad, pv_banks[half])
                    for gc in range(4):
                        tp = ps_m.tile([128, 128], f32, tag="m")
                        nc.tensor.transpose(
                            tp, uT_pad[:, gc * 128 : (gc + 1) * 128], ident
                        )
                        nc.vector.tensor_copy(
                            u_nat[:, gc, half * 128 : (half + 1) * 128], tp
                        )
                u_blk = u_nat.rearrange("p c (h x) -> p c h x", x=32)
                recip8 = pmisc.tile([128, 4, 8, 1], f32)
                nc.vector.reciprocal(recip8, u_blk[:, :, :, 16:17])
                u_c = pmisc.tile([128, 4, 128], f32)
                for gc in range(4):
                    for h in range(8):
                        nc.vector.tensor_scalar_mul(
                            u_c[:, gc, 16 * h : 16 * h + 16],
                            u_blk[:, gc, h, 0:16],
                            recip8[:, gc, h, :],
                        )
                uT_norm = pmisc.tile([128, 512], f32)
                for gc in range(4):
                    tp = ps_m.tile([128, 128], f32, tag="m")
                    nc.tensor.transpose(tp, u_c[:, gc, :], ident)
                    nc.vector.tensor_copy(uT_norm[:, gc * 128 : (gc + 1) * 128], tp)

                # ---- combine + bias ----
                mh_ps = ps_m.tile([128, 512], f32, tag="m")
                nc.tensor.matmul(mh_ps, w_sb["Wc"], uT_norm, start=True, stop=True)
                mhT = pmisc.tile([128, 512], f32)
                nc.vector.tensor_scalar_add(mhT, mh_ps, b_sb[:, 0:1])

                # ---- branch 2: single-head scoring, natural layout ----
                tanh_sb = pstage.tile([128, 4, 512], f32)
                for gc in range(4):
                    s2 = ps_m.tile([128, 512], f32, tag="m")
                    nc.tensor.matmul(
                        s2,
                        mhT[:, gc * 128 : (gc + 1) * 128],
                        nodesT,
                        start=True,
                        stop=True,
                    )
                    nc.scalar.activation(
                        tanh_sb[:, gc, :], s2, AF.Tanh, scale=1.0 / SQRT_E
                    )
                # z = tanh + mask (mask scale-invariant under the x10)
                nc.vector.tensor_add(tanh_sb, tanh_sb, mask_t)
                den = pmisc.tile([128, 4], f32)
                p_sb = pstage.tile([128, 4, 512], f32)
                for gc in range(4):
                    nc.scalar.activation(
                        p_sb[:, gc, :],
                        tanh_sb[:, gc, :],
                        AF.Exp,
                        scale=10.0,
                        accum_out=den[:, gc : gc + 1],
                    )
                recipden = pmisc.tile([128, 4], f32)
                nc.vector.reciprocal(recipden, den)
                stage = pstage.tile([128, 4, 512], f32)
                for gc in range(4):
                    nc.vector.tensor_scalar_mul(
                        stage[:, gc, :], p_sb[:, gc, :], recipden[:, gc : gc + 1]
                    )
                nc.sync.dma_start(
                    out=probs_d[b].rearrange("(c p) n -> p c n", p=128), in_=stage
                )

    if legalize:
        _legalize_waits(nc)
    return nc


def _prep_weights(inputs):
    def pad4(W):
        Wp0 = np.zeros((E, 128), np.float32)
        Wp1 = np.zeros((E, 128), np.float32)
        for j in range(4):
            Wp0[:, 32 * j : 32 * j + 16] = W[:, 16 * j : 16 * j + 16]
            Wp1[:, 32 * j : 32 * j + 16] = W[:, 64 + 16 * j : 64 + 16 * j + 16]
        return Wp0, Wp1

    s = np.float32(1.0 / np.sqrt(np.float32(D)))  # 0.25 folded into q weights
    Wq1p0, Wq1p1 = pad4(np.asarray(inputs["Wq_first"], np.float32) * s)
    Wqlp0, Wqlp1 = pad4(np.asarray(inputs["Wq_last"], np.float32) * s)
    Wkp0, Wkp1 = pad4(np.asarray(inputs["Wk"], np.float32))
    return {
        "Wq1p0": Wq1p0,
        "Wq1p1": Wq1p1,
        "Wqlp0": Wqlp0,
        "Wqlp1": Wqlp1,
        "Wkp0": Wkp0,
        "Wkp1": Wkp1,
        "Wv": np.ascontiguousarray(np.asarray(inputs["Wv"], np.float32)),
        "Wc": np.ascontiguousarray(np.asarray(inputs["W_comb"], np.float32)),
        "bc": np.asarray(inputs["b_comb"], np.float32).reshape(E, 1),
    }


def run(inputs, trace=False):
    from concourse.bass_utils import run_bass_kernel_spmd

    if "nc" not in _CACHE:
        _CACHE["nc"] = _build_nc()
    nc = _CACHE["nc"]

    w = _prep_weights(inputs)
    nodes = np.ascontiguousarray(np.asarray(inputs["encoded_nodes"], np.float32))
    q1 = np.ascontiguousarray(np.asarray(inputs["encoded_q1"], np.float32))
    last = np.ascontiguousarray(np.asarray(inputs["encoded_last_node"], np.float32))
    mask = np.ascontiguousarray(np.asarray(inputs["ninf_mask"], np.float32))

    in_maps = []
    for i in range(NCORES):
        sl = slice(i * BL, (i + 1) * BL)
        in_maps.append(
            {
                "nodes": nodes[sl],
                "q1": q1[sl],
                "last": last[sl],
                "mask": mask[sl],
                **w,
            }
        )
    res = run_bass_kernel_spmd(nc, in_maps, list(range(NCORES)), trace=trace)
    out = np.concatenate([res.results[i]["probs"] for i in range(NCORES)], axis=0)
    return out, res


def kernel(**inputs) -> np.ndarray:
    out, _ = run(inputs, trace=False)
    return out


# revision 14
# speedup vs baseline: 572.8175x; 572.8175x over previous
"""Trainium2 Bass kernel for nn_Decoder_14680198217759.

Multi-head attention decoder (B=32, G=N=512, E=128, H=8, D=16), pure data
parallel over 8 NeuronCores (4 batches/core), fp32 throughout.

Layout strategy per batch (all on one core):
  - Activations transposed via PE so E sits on partitions: xT [E, G].
  - Projections produce qT/kT in a head-padded layout (head j of a 4-head
    group at partition offset 32j) so per-head K=16 score matmuls can use
    legal base partitions and 2-way PE row tiling.
  - Scores computed TRANSPOSED: scoreT_h [n, g] (head pair packed into one
    [128, 1024] PSUM tile), so the softmax(n) contraction of PV needs no
    transpose of the big prob matrix.
  - The rank-3 mask rides the PE: maskT is added into the score PSUM via an
    identity-matmul (lhsT = natural mask chunk, rhs = I) per head.
  - exp on ACT (PSUM -> SBUF). PV contracts exp with v_aug (v plus a ones
    column) giving out^T and the softmax denominators in one pass.
  - Per-head normalization happens in natural space: PV output transposed
    back [g, hd], reciprocal on the strided ones columns, per-partition
    tensor_scalar multiplies, transpose again for the combine matmul.
  - Branch 2 (single-head scoring) stays natural [g, n]: tanh on ACT with
    input scale 1/sqrt(E); mask added with one DVE tensor_tensor (the mask is
    scale invariant: 10*(t + m) masks as well as 10 t + m); exp with scale=10
    and accum_out producing the denominators for free.
"""

import numpy as np

B, G, N, E, H, D = 32, 512, 512, 128, 8, 16
SQRT_E = 11.313708498984761
NCORES = 8
BL = B // NCORES  # batches per core

_CACHE = {}


# --------------------------------------------------------------------------
# BIR wait legalization: this toolchain's walrus accepts at most ONE sem wait
# per instruction; Tile's scheduler can emit more (notably on the kernel-tail
# drain). Split excess waits onto same-engine NoOps placed directly before
# the offending instruction (same-queue program order keeps the semantics).
# --------------------------------------------------------------------------
def _legalize_waits(nc, max_waits=1):
    import concourse.mybir as mybir

    n_split = 0
    for f in nc.m.functions:
        for bb in f.blocks:
            out = []
            for ins in bb.instructions:
                si = ins.sync_info
                waits = list(si.on_wait) if si and si.on_wait else []
                if len(waits) > max_waits:
                    while len(waits) > max_waits:
                        chunk, waits = waits[:max_waits], waits[max_waits:]
                        nop = mybir.InstNoOp(
                            name=f"I-waitfix-{nc.next_id()}", ins=[], outs=[]
                        )
                        nop.engine = ins.engine
                        nop.sync_info = mybir.SyncInfo(on_wait=chunk, on_update=[])
                        out.append(nop)
                        n_split += 1
                    ins.sync_info = mybir.SyncInfo(
                        on_wait=waits, on_update=list(si.on_update or [])
                    )
                out.append(ins)
            bb.instructions[:] = out
    return n_split


def _build_nc(legalize=True):
    import concourse.bass as bass
    import concourse.mybir as mybir
    import concourse.tile as tile
    from concourse.masks import make_identity

    f32 = mybir.dt.float32
    f32r = mybir.dt.float32r  # PE fast fp32 path (1 cycle/row vs 4)
    bf16 = mybir.dt.bfloat16
    AF = mybir.ActivationFunctionType

    nc = bass.Bass()

    nodes_d = nc.dram_tensor("nodes", [BL, N, E], f32, kind="ExternalInput")
    q1_d = nc.dram_tensor("q1", [BL, G, E], f32, kind="ExternalInput")
    last_d = nc.dram_tensor("last", [BL, G, E], f32, kind="ExternalInput")
    mask_d = nc.dram_tensor("mask", [BL, G, N], f32, kind="ExternalInput")
    wnames = ["Wq1p0", "Wq1p1", "Wqlp0", "Wqlp1", "Wkp0", "Wkp1", "Wv", "Wc"]
    w_d = {n: nc.dram_tensor(n, [E, 128], f32, kind="ExternalInput") for n in wnames}
    b_d = nc.dram_tensor("bc", [E, 1], f32, kind="ExternalInput")
    probs_d = nc.dram_tensor("probs", [BL, G, N], f32, kind="ExternalOutput")

    with tile.TileContext(nc) as tc:
        import contextlib

        with contextlib.ExitStack() as ctx:
            pw = ctx.enter_context(tc.tile_pool(name="pw", bufs=1))
            pin = ctx.enter_context(tc.tile_pool(name="pin", bufs=2))
            pxt = ctx.enter_context(tc.tile_pool(name="pxt", bufs=2))
            pproj = ctx.enter_context(tc.tile_pool(name="pproj", bufs=2))
            pexp = ctx.enter_context(tc.tile_pool(name="pexp", bufs=6))
            pmisc = ctx.enter_context(tc.tile_pool(name="pmisc", bufs=2))
            pstage = ctx.enter_context(tc.tile_pool(name="pstage", bufs=2))
            ps_score = ctx.enter_context(
                tc.tile_pool(name="ps_score", bufs=2, space="PSUM")
            )
            ps_pv = ctx.enter_context(tc.tile_pool(name="ps_pv", bufs=2, space="PSUM"))
            ps_m = ctx.enter_context(tc.tile_pool(name="ps_m", bufs=2, space="PSUM"))

            # ---- constants / weights (once) ----
            ident = pw.tile([128, 128], f32)
            make_identity(nc, ident)
            ident_b = pw.tile([128, 128], bf16)
            make_identity(nc, ident_b)
            # weights: DMA raw fp32, then one-time round to f32r for the PE
            w_sb = {}
            for n in wnames:
                w_raw = pw.tile([128, 128], f32, name=f"wr_{n}", tag=f"wr_{n}")
                nc.sync.dma_start(out=w_raw, in_=w_d[n][:, :])
                w_sb[n] = pw.tile([128, 128], f32r, name=f"w_{n}", tag=f"w_{n}")
                nc.vector.tensor_copy(w_sb[n], w_raw)
            b_sb = pw.tile([128, 1], f32)
            nc.sync.dma_start(out=b_sb, in_=b_d[:, :])
            # v_aug: per n-chunk, 8 heads at 32-col blocks: cols 32h..32h+15 =
            # v head h, col 32h+16 = 1.0 (denominator row), rest zero.
            v_aug = pw.tile([128, 4, 256], bf16)
            nc.vector.memset(v_aug, 0.0)
            v_aug_blk = v_aug.rearrange("p c (h x) -> p c h x", x=32)
            nc.vector.memset(v_aug_blk[:, :, :, 16:17], 1.0)

            for b in range(BL):
                # ---- loads ----
                x_nodes = pin.tile([128, 4, 128], f32)
                nc.sync.dma_start(
                    out=x_nodes, in_=nodes_d[b].rearrange("(c p) e -> p c e", p=128)
                )
                x_q1 = pin.tile([128, 4, 128], f32)
                nc.sync.dma_start(
                    out=x_q1, in_=q1_d[b].rearrange("(c p) e -> p c e", p=128)
                )
                x_last = pin.tile([128, 4, 128], f32)
                nc.sync.dma_start(
                    out=x_last, in_=last_d[b].rearrange("(c p) e -> p c e", p=128)
                )
                mask_t = pin.tile([128, 4, 512], bf16)
                nc.gpsimd.dma_start(
                    out=mask_t, in_=mask_d[b].rearrange("(c p) n -> p c n", p=128)
                )

                # ---- transpose activations to [E, G] ----
                def transpose_to(dst, src_nat):
                    for c in range(4):
                        tp = ps_m.tile([128, 128], f32, tag="m")
                        nc.tensor.transpose(tp, src_nat[:, c, :], ident)
                        nc.vector.tensor_copy(dst[:, c * 128 : (c + 1) * 128], tp)

                nodesT = pxt.tile([128, 512], f32r)
                transpose_to(nodesT, x_nodes)
                q1T = pxt.tile([128, 512], f32r)
                transpose_to(q1T, x_q1)
                lastT = pxt.tile([128, 512], f32r)
                transpose_to(lastT, x_last)

                # ---- projections (outputs transposed/padded) ----
                qT = []
                for g4 in range(2):  # head groups 0-3 / 4-7
                    ps = ps_m.tile([128, 512], f32, tag="m")
                    nc.tensor.matmul(
                        ps, w_sb[f"Wq1p{g4}"], q1T, start=True, stop=False
                    )
                    nc.tensor.matmul(
                        ps, w_sb[f"Wqlp{g4}"], lastT, start=False, stop=True
                    )
                    t = pproj.tile([128, 512], f32r, tag=f"qT{g4}")
                    nc.vector.tensor_copy(t, ps)
                    qT.append(t)
                kT = []
                for g4 in range(2):
                    ps = ps_m.tile([128, 512], f32, tag="m")
                    nc.tensor.matmul(
                        ps, w_sb[f"Wkp{g4}"], nodesT, start=True, stop=True
                    )
                    t = pproj.tile([128, 512], f32r, tag=f"kT{g4}")
                    nc.vector.tensor_copy(t, ps)
                    kT.append(t)
                # v natural [n, hd] scattered into v_aug 32-blocks
                for c in range(4):
                    ps = ps_m.tile([128, 128], f32, tag="m")
                    nc.tensor.matmul(
                        ps,
                        nodesT[:, c * 128 : (c + 1) * 128],
                        w_sb["Wv"],
                        start=True,
                        stop=True,
                    )
                    nc.vector.tensor_copy(
                        v_aug_blk[:, c, :, 0:16],
                        ps.rearrange("p (h d) -> p h d", d=16),
                    )

                # ---- attention: scores (transposed) + exp + PV ----
                pv_banks = []
                for half in range(2):  # heads 0-3 then 4-7
                    pv = ps_pv.tile([128, 512], f32, tag="pv")
                    pv_banks.append(pv)
                    for hp in range(2):  # head pair within group
                        j0, j1 = 2 * hp, 2 * hp + 1  # pad-slot indices
                        expps = []
                        for c in range(4):  # n-chunks
                            sc = ps_score.tile([128, 1024], f32, tag="sc")
                            for idx, j in enumerate((j0, j1)):
                                nc.tensor.matmul(
                                    sc[:, idx * 512 : (idx + 1) * 512],
                                    kT[half][
                                        32 * j : 32 * j + 16,
                                        c * 128 : (c + 1) * 128,
                                    ],
                                    qT[half][32 * j : 32 * j + 16, :],
                                    start=True,
                                    stop=False,
                                    tile_position=(32 * j, 0),
                                )
                            # += maskT via identity trick, per head.
                            # gc outer so consecutive MMs share the same
                            # stationary mask chunk (one weight load, 2 MMs).
                            for gc in range(4):
                                for idx in range(2):
                                    nc.tensor.matmul(
                                        sc[
                                            :,
                                            idx * 512
                                            + gc * 128 : idx * 512
                                            + (gc + 1) * 128,
                                        ],
                                        mask_t[:, gc, c * 128 : (c + 1) * 128],
                                        ident_b,
                                        start=False,
                                        stop=(gc == 3),
                                    )
                            expp = pexp.tile([128, 1024], bf16, tag="expp")
                            nc.scalar.activation(expp, sc, AF.Exp)
                            expps.append(expp)
                        # PV accumulate: out_aug^T rows 32j. The psum group
                        # checker requires groups in one bank to be strictly
                        # sequential, so each head's 4 n-chunk MMs run
                        # back-to-back as one start..stop group.
                        for idx, j in enumerate((j0, j1)):
                            vcol = 32 * (half * 4 + j)  # head's v_aug block
                            for c in range(4):
                                nc.tensor.matmul(
                                    pv[32 * j : 32 * j + 32, :],
                                    v_aug[:, c, vcol : vcol + 32],
                                    expps[c][:, idx * 512 : (idx + 1) * 512],
                                    start=(c == 0),
                                    stop=(c == 3),
                                    tile_position=(0, 32 * j),
                                )

                # ---- normalize per head, in natural space ----
                u_nat = pmisc.tile([128, 4, 256], f32)
                for half in range(2):
                    uT_pad = pmisc.tile([128, 512], f32, tag="uT_pad")
                    nc.vector.tensor_copy(uT_pad, pv_banks[half])
                    for gc in range(4):
                        tp = ps_m.tile([128, 128], f32, tag="m")
                        nc.tensor.transpose(
                            tp, uT_pad[:, gc * 128 : (gc + 1) * 128], ident
                        )
                        nc.vector.tensor_copy(
                            u_nat[:, gc, half * 128 : (half + 1) * 128], tp
                        )
                u_blk = u_nat.rearrange("p c (h x) -> p c h x", x=32)
                recip8 = pmisc.tile([128, 4, 8, 1], f32)
                nc.vector.reciprocal(recip8, u_blk[:, :, :, 16:17])
                u_c = pmisc.tile([128, 4, 128], f32)
                for gc in range(4):
                    for h in range(8):
                        nc.vector.tensor_scalar_mul(
                            u_c[:, gc, 16 * h : 16 * h + 16],
                            u_blk[:, gc, h, 0:16],
                            recip8[:, gc, h, :],
                        )
                uT_norm = pmisc.tile([128, 512], f32r)
                for gc in range(4):
                    tp = ps_m.tile([128, 128], f32, tag="m")
                    nc.tensor.transpose(tp, u_c[:, gc, :], ident)
                    nc.vector.tensor_copy(uT_norm[:, gc * 128 : (gc + 1) * 128], tp)

                # ---- combine + bias ----
                mh_ps = ps_m.tile([128, 512], f32, tag="m")
                nc.tensor.matmul(
                    mh_ps, w_sb["Wc"], uT_norm, start=True, stop=True
                )
                mhT = pmisc.tile([128, 512], f32r)
                nc.vector.tensor_scalar_add(mhT, mh_ps, b_sb[:, 0:1])

                # ---- branch 2: single-head scoring, natural layout ----
                tanh_sb = pstage.tile([128, 4, 512], f32)
                for gc in range(4):
                    s2 = ps_m.tile([128, 512], f32, tag="m")
                    nc.tensor.matmul(
                        s2,
                        mhT[:, gc * 128 : (gc + 1) * 128],
                        nodesT,
                        start=True,
                        stop=True,
                    )
                    nc.scalar.activation(
                        tanh_sb[:, gc, :], s2, AF.Tanh, scale=1.0 / SQRT_E
                    )
                # z = tanh + mask (mask scale-invariant under the x10)
                nc.vector.tensor_add(tanh_sb, tanh_sb, mask_t)
                den = pmisc.tile([128, 4], f32)
                p_sb = pstage.tile([128, 4, 512], f32)
                for gc in range(4):
                    nc.scalar.activation(
                        p_sb[:, gc, :],
                        tanh_sb[:, gc, :],
                        AF.Exp,
                        scale=10.0,
                        accum_out=den[:, gc : gc + 1],
                    )
                recipden = pmisc.tile([128, 4], f32)
                nc.vector.reciprocal(recipden, den)
                stage = pstage.tile([128, 4, 512], f32)
                for gc in range(4):
                    nc.vector.tensor_scalar_mul(
                        stage[:, gc, :], p_sb[:, gc, :], recipden[:, gc : gc + 1]
                    )
                nc.sync.dma_start(
                    out=probs_d[b].rearrange("(c p) n -> p c n", p=128), in_=stage
                )

    if legalize:
        _legalize_waits(nc)
    return nc


def _prep_weights(inputs):
    def pad4(W):
        Wp0 = np.zeros((E, 128), np.float32)
        Wp1 = np.zeros((E, 128), np.float32)
        for j in range(4):
            Wp0[:, 32 * j : 32 * j + 16] = W[:, 16 * j : 16 * j + 16]
            Wp1[:, 32 * j : 32 * j + 16] = W[:, 64 + 16 * j : 64 + 16 * j + 16]
        return Wp0, Wp1

    s = np.float32(1.0 / np.sqrt(np.float32(D)))  # 0.25 folded into q weights
    Wq1p0, Wq1p1 = pad4(np.asarray(inputs["Wq_first"], np.float32) * s)
    Wqlp0, Wqlp1 = pad4(np.asarray(inputs["Wq_last"], np.float32) * s)
    Wkp0, Wkp1 = pad4(np.asarray(inputs["Wk"], np.float32))
    return {
        "Wq1p0": Wq1p0,
        "Wq1p1": Wq1p1,
        "Wqlp0": Wqlp0,
        "Wqlp1": Wqlp1,
        "Wkp0": Wkp0,
        "Wkp1": Wkp1,
        "Wv": np.ascontiguousarray(np.asarray(inputs["Wv"], np.float32)),
        "Wc": np.ascontiguousarray(np.asarray(inputs["W_comb"], np.float32)),
        "bc": np.asarray(inputs["b_comb"], np.float32).reshape(E, 1),
    }


def run(inputs, trace=False):
    from concourse.bass_utils import run_bass_kernel_spmd

    if "nc" not in _CACHE:
        _CACHE["nc"] = _build_nc()
    nc = _CACHE["nc"]

    w = _prep_weights(inputs)
    nodes = np.ascontiguousarray(np.asarray(inputs["encoded_nodes"], np.float32))
    q1 = np.ascontiguousarray(np.asarray(inputs["encoded_q1"], np.float32))
    last = np.ascontiguousarray(np.asarray(inputs["encoded_last_node"], np.float32))
    mask = np.ascontiguousarray(np.asarray(inputs["ninf_mask"], np.float32))

    in_maps = []
    for i in range(NCORES):
        sl = slice(i * BL, (i + 1) * BL)
        in_maps.append(
            {
                "nodes": nodes[sl],
                "q1": q1[sl],
                "last": last[sl],
                "mask": mask[sl],
                **w,
            }
        )
    try:
        res = run_bass_kernel_spmd(nc, in_maps, list(range(NCORES)), trace=trace)
    except Exception:
        # The first execution of a freshly compiled NEFF occasionally dies
        # with NRT_EXEC_UNIT_UNRECOVERABLE on this stack; a retry with the
        # cached NEFF has always succeeded.
        res = run_bass_kernel_spmd(nc, in_maps, list(range(NCORES)), trace=trace)
    out = np.concatenate([res.results[i]["probs"] for i in range(NCORES)], axis=0)
    return out, res


def kernel(**inputs) -> np.ndarray:
    out, _ = run(inputs, trace=False)
    return out


# revision 15
# speedup vs baseline: 609.7024x; 1.0644x over previous
"""Trainium2 Bass kernel for nn_Decoder_14680198217759.

Multi-head attention decoder (B=32, G=N=512, E=128, H=8, D=16), pure data
parallel over 8 NeuronCores (4 batches/core), fp32 throughout.

Layout strategy per batch (all on one core):
  - Activations transposed via PE so E sits on partitions: xT [E, G].
  - Projections produce qT/kT in a head-padded layout (head j of a 4-head
    group at partition offset 32j) so per-head K=16 score matmuls can use
    legal base partitions and 2-way PE row tiling.
  - Scores computed TRANSPOSED: scoreT_h [n, g] (head pair packed into one
    [128, 1024] PSUM tile), so the softmax(n) contraction of PV needs no
    transpose of the big prob matrix.
  - The rank-3 mask rides the PE: maskT is added into the score PSUM via an
    identity-matmul (lhsT = natural mask chunk, rhs = I) per head.
  - exp on ACT (PSUM -> SBUF). PV contracts exp with v_aug (v plus a ones
    column) giving out^T and the softmax denominators in one pass.
  - Per-head normalization happens in natural space: PV output transposed
    back [g, hd], reciprocal on the strided ones columns, per-partition
    tensor_scalar multiplies, transpose again for the combine matmul.
  - Branch 2 (single-head scoring) stays natural [g, n]: tanh on ACT with
    input scale 1/sqrt(E); mask added with one DVE tensor_tensor (the mask is
    scale invariant: 10*(t + m) masks as well as 10 t + m); exp with scale=10
    and accum_out producing the denominators for free.
"""

import numpy as np

B, G, N, E, H, D = 32, 512, 512, 128, 8, 16
SQRT_E = 11.313708498984761
NCORES = 8
BL = B // NCORES  # batches per core

_CACHE = {}


# --------------------------------------------------------------------------
# BIR wait legalization: this toolchain's walrus accepts at most ONE sem wait
# per instruction; Tile's scheduler can emit more (notably on the kernel-tail
# drain). Split excess waits onto same-engine NoOps placed directly before
# the offending instruction (same-queue program order keeps the semantics).
# --------------------------------------------------------------------------
def _legalize_waits(nc, max_waits=1):
    import concourse.mybir as mybir

    n_split = 0
    for f in nc.m.functions:
        for bb in f.blocks:
            out = []
            for ins in bb.instructions:
                si = ins.sync_info
                waits = list(si.on_wait) if si and si.on_wait else []
                if len(waits) > max_waits:
                    while len(waits) > max_waits:
                        chunk, waits = waits[:max_waits], waits[max_waits:]
                        nop = mybir.InstNoOp(
                            name=f"I-waitfix-{nc.next_id()}", ins=[], outs=[]
                        )
                        nop.engine = ins.engine
                        nop.sync_info = mybir.SyncInfo(on_wait=chunk, on_update=[])
                        out.append(nop)
                        n_split += 1
                    ins.sync_info = mybir.SyncInfo(
                        on_wait=waits, on_update=list(si.on_update or [])
                    )
                out.append(ins)
            bb.instructions[:] = out
    return n_split


def _build_nc(legalize=True):
    import concourse.bass as bass
    import concourse.mybir as mybir
    import concourse.tile as tile
    from concourse.masks import make_identity

    f32 = mybir.dt.float32
    f32r = mybir.dt.float32r  # PE fast fp32 path (1 cycle/row vs 4)
    bf16 = mybir.dt.bfloat16
    AF = mybir.ActivationFunctionType

    nc = bass.Bass()

    nodes_d = nc.dram_tensor("nodes", [BL, N, E], f32, kind="ExternalInput")
    q1_d = nc.dram_tensor("q1", [BL, G, E], f32, kind="ExternalInput")
    last_d = nc.dram_tensor("last", [BL, G, E], f32, kind="ExternalInput")
    mask_d = nc.dram_tensor("mask", [BL, G, N], f32, kind="ExternalInput")
    wnames = ["Wq1p0", "Wq1p1", "Wqlp0", "Wqlp1", "Wkp0", "Wkp1", "Wv", "Wc"]
    w_d = {n: nc.dram_tensor(n, [E, 128], f32, kind="ExternalInput") for n in wnames}
    b_d = nc.dram_tensor("bc", [E, 1], f32, kind="ExternalInput")
    probs_d = nc.dram_tensor("probs", [BL, G, N], f32, kind="ExternalOutput")

    with tile.TileContext(nc) as tc:
        import contextlib

        with contextlib.ExitStack() as ctx:
            pw = ctx.enter_context(tc.tile_pool(name="pw", bufs=1))
            pin = ctx.enter_context(tc.tile_pool(name="pin", bufs=3))
            pxt = ctx.enter_context(tc.tile_pool(name="pxt", bufs=3))
            pproj = ctx.enter_context(tc.tile_pool(name="pproj", bufs=3))
            pexp = ctx.enter_context(tc.tile_pool(name="pexp", bufs=6))
            pmisc = ctx.enter_context(tc.tile_pool(name="pmisc", bufs=3))
            pstage = ctx.enter_context(tc.tile_pool(name="pstage", bufs=3))
            ps_score = ctx.enter_context(
                tc.tile_pool(name="ps_score", bufs=2, space="PSUM")
            )
            ps_pv = ctx.enter_context(tc.tile_pool(name="ps_pv", bufs=2, space="PSUM"))
            ps_m = ctx.enter_context(tc.tile_pool(name="ps_m", bufs=2, space="PSUM"))

            # ---- constants / weights (once) ----
            ident = pw.tile([128, 128], f32)
            make_identity(nc, ident)
            ident_b = pw.tile([128, 128], bf16)
            make_identity(nc, ident_b)
            # weights: DMA raw fp32, then one-time round to f32r for the PE
            w_sb = {}
            for n in wnames:
                w_raw = pw.tile([128, 128], f32, name=f"wr_{n}", tag=f"wr_{n}")
                nc.sync.dma_start(out=w_raw, in_=w_d[n][:, :])
                w_sb[n] = pw.tile([128, 128], f32r, name=f"w_{n}", tag=f"w_{n}")
                nc.vector.tensor_copy(w_sb[n], w_raw)
            b_sb = pw.tile([128, 1], f32)
            nc.sync.dma_start(out=b_sb, in_=b_d[:, :])
            # v_aug: per n-chunk, 8 heads at 32-col blocks: cols 32h..32h+15 =
            # v head h, col 32h+16 = 1.0 (denominator row), rest zero.
            v_aug = pw.tile([128, 4, 256], bf16)
            nc.vector.memset(v_aug, 0.0)
            v_aug_blk = v_aug.rearrange("p c (h x) -> p c h x", x=32)
            nc.vector.memset(v_aug_blk[:, :, :, 16:17], 1.0)

            for b in range(BL):
                # ---- loads ----
                x_nodes = pin.tile([128, 4, 128], f32)
                nc.sync.dma_start(
                    out=x_nodes, in_=nodes_d[b].rearrange("(c p) e -> p c e", p=128)
                )
                x_q1 = pin.tile([128, 4, 128], f32)
                nc.sync.dma_start(
                    out=x_q1, in_=q1_d[b].rearrange("(c p) e -> p c e", p=128)
                )
                x_last = pin.tile([128, 4, 128], f32)
                nc.sync.dma_start(
                    out=x_last, in_=last_d[b].rearrange("(c p) e -> p c e", p=128)
                )
                mask_t = pin.tile([128, 4, 512], bf16)
                nc.gpsimd.dma_start(
                    out=mask_t, in_=mask_d[b].rearrange("(c p) n -> p c n", p=128)
                )

                # ---- transpose activations to [E, G] ----
                def transpose_to(dst, src_nat):
                    for c in range(4):
                        tp = ps_m.tile([128, 128], f32, tag="m")
                        nc.tensor.transpose(tp, src_nat[:, c, :], ident)
                        nc.vector.tensor_copy(dst[:, c * 128 : (c + 1) * 128], tp)

                nodesT = pxt.tile([128, 512], f32r)
                transpose_to(nodesT, x_nodes)
                q1T = pxt.tile([128, 512], f32r)
                transpose_to(q1T, x_q1)
                lastT = pxt.tile([128, 512], f32r)
                transpose_to(lastT, x_last)

                # ---- projections (outputs transposed/padded) ----
                qT = []
                for g4 in range(2):  # head groups 0-3 / 4-7
                    ps = ps_m.tile([128, 512], f32, tag="m")
                    nc.tensor.matmul(
                        ps, w_sb[f"Wq1p{g4}"], q1T, start=True, stop=False
                    )
                    nc.tensor.matmul(
                        ps, w_sb[f"Wqlp{g4}"], lastT, start=False, stop=True
                    )
                    t = pproj.tile([128, 512], f32r, tag=f"qT{g4}")
                    nc.vector.tensor_copy(t, ps)
                    qT.append(t)
                kT = []
                for g4 in range(2):
                    ps = ps_m.tile([128, 512], f32, tag="m")
                    nc.tensor.matmul(
                        ps, w_sb[f"Wkp{g4}"], nodesT, start=True, stop=True
                    )
                    t = pproj.tile([128, 512], f32r, tag=f"kT{g4}")
                    nc.vector.tensor_copy(t, ps)
                    kT.append(t)
                # v natural [n, hd] scattered into v_aug 32-blocks
                for c in range(4):
                    ps = ps_m.tile([128, 128], f32, tag="m")
                    nc.tensor.matmul(
                        ps,
                        nodesT[:, c * 128 : (c + 1) * 128],
                        w_sb["Wv"],
                        start=True,
                        stop=True,
                    )
                    nc.vector.tensor_copy(
                        v_aug_blk[:, c, :, 0:16],
                        ps.rearrange("p (h d) -> p h d", d=16),
                    )

                # ---- attention: scores (transposed) + exp + PV ----
                pv_banks = []
                for half in range(2):  # heads 0-3 then 4-7
                    pv = ps_pv.tile([128, 512], f32, tag="pv")
                    pv_banks.append(pv)
                    for hp in range(2):  # head pair within group
                        j0, j1 = 2 * hp, 2 * hp + 1  # pad-slot indices
                        expps = []
                        for c in range(4):  # n-chunks
                            sc = ps_score.tile([128, 1024], f32, tag="sc")
                            for idx, j in enumerate((j0, j1)):
                                nc.tensor.matmul(
                                    sc[:, idx * 512 : (idx + 1) * 512],
                                    kT[half][
                                        32 * j : 32 * j + 16,
                                        c * 128 : (c + 1) * 128,
                                    ],
                                    qT[half][32 * j : 32 * j + 16, :],
                                    start=True,
                                    stop=False,
                                    tile_position=(32 * j, 0),
                                )
                            # += maskT via identity trick, per head.
                            # gc outer so consecutive MMs share the same
                            # stationary mask chunk (one weight load, 2 MMs).
                            for gc in range(4):
                                for idx in range(2):
                                    nc.tensor.matmul(
                                        sc[
                                            :,
                                            idx * 512
                                            + gc * 128 : idx * 512
                                            + (gc + 1) * 128,
                                        ],
                                        mask_t[:, gc, c * 128 : (c + 1) * 128],
                                        ident_b,
                                        start=False,
                                        stop=(gc == 3),
                                    )
                            expp = pexp.tile([128, 1024], bf16, tag="expp")
                            nc.scalar.activation(expp, sc, AF.Exp)
                            expps.append(expp)
                        # PV accumulate: out_aug^T rows 32j. The psum group
                        # checker requires groups in one bank to be strictly
                        # sequential, so each head's 4 n-chunk MMs run
                        # back-to-back as one start..stop group.
                        for idx, j in enumerate((j0, j1)):
                            vcol = 32 * (half * 4 + j)  # head's v_aug block
                            for c in range(4):
                                nc.tensor.matmul(
                                    pv[32 * j : 32 * j + 32, :],
                                    v_aug[:, c, vcol : vcol + 32],
                                    expps[c][:, idx * 512 : (idx + 1) * 512],
                                    start=(c == 0),
                                    stop=(c == 3),
                                    tile_position=(0, 32 * j),
                                )

                # ---- normalize per head, in natural space ----
                u_nat = pmisc.tile([128, 4, 256], f32)
                for half in range(2):
                    uT_pad = pmisc.tile([128, 512], f32, tag="uT_pad")
                    nc.vector.tensor_copy(uT_pad, pv_banks[half])
                    for gc in range(4):
                        tp = ps_m.tile([128, 128], f32, tag="m")
                        nc.tensor.transpose(
                            tp, uT_pad[:, gc * 128 : (gc + 1) * 128], ident
                        )
                        nc.vector.tensor_copy(
                            u_nat[:, gc, half * 128 : (half + 1) * 128], tp
                        )
                u_blk = u_nat.rearrange("p c (h x) -> p c h x", x=32)
                recip8 = pmisc.tile([128, 4, 8, 1], f32)
                nc.vector.reciprocal(recip8, u_blk[:, :, :, 16:17])
                u_c = pmisc.tile([128, 4, 128], f32)
                for gc in range(4):
                    for h in range(8):
                        nc.vector.tensor_scalar_mul(
                            u_c[:, gc, 16 * h : 16 * h + 16],
                            u_blk[:, gc, h, 0:16],
                            recip8[:, gc, h, :],
                        )
                uT_norm = pmisc.tile([128, 512], f32r)
                for gc in range(4):
                    tp = ps_m.tile([128, 128], f32, tag="m")
                    nc.tensor.transpose(tp, u_c[:, gc, :], ident)
                    nc.vector.tensor_copy(uT_norm[:, gc * 128 : (gc + 1) * 128], tp)

                # ---- combine + bias ----
                mh_ps = ps_m.tile([128, 512], f32, tag="m")
                nc.tensor.matmul(
                    mh_ps, w_sb["Wc"], uT_norm, start=True, stop=True
                )
                mhT = pmisc.tile([128, 512], f32r)
                nc.vector.tensor_scalar_add(mhT, mh_ps, b_sb[:, 0:1])

                # ---- branch 2: single-head scoring, natural layout ----
                tanh_sb = pstage.tile([128, 4, 512], f32)
                for gc in range(4):
                    s2 = ps_m.tile([128, 512], f32, tag="m")
                    nc.tensor.matmul(
                        s2,
                        mhT[:, gc * 128 : (gc + 1) * 128],
                        nodesT,
                        start=True,
                        stop=True,
                    )
                    nc.scalar.activation(
                        tanh_sb[:, gc, :], s2, AF.Tanh, scale=1.0 / SQRT_E
                    )
                # z = tanh + mask (mask scale-invariant under the x10)
                nc.gpsimd.tensor_add(tanh_sb, tanh_sb, mask_t)
                den = pmisc.tile([128, 4], f32)
                p_sb = pstage.tile([128, 4, 512], f32)
                for gc in range(4):
                    nc.scalar.activation(
                        p_sb[:, gc, :],
                        tanh_sb[:, gc, :],
                        AF.Exp,
                        scale=10.0,
                        accum_out=den[:, gc : gc + 1],
                    )
                recipden = pmisc.tile([128, 4], f32)
                nc.vector.reciprocal(recipden, den)
                stage = pstage.tile([128, 4, 512], f32)
                for gc in range(4):
                    nc.vector.tensor_scalar_mul(
                        stage[:, gc, :], p_sb[:, gc, :], recipden[:, gc : gc + 1]
                    )
                nc.sync.dma_start(
                    out=probs_d[b].rearrange("(c p) n -> p c n", p=128), in_=stage
                )

    if legalize:
        _legalize_waits(nc)
    return nc


def _prep_weights(inputs):
    def pad4(W):
        Wp0 = np.zeros((E, 128), np.float32)
        Wp1 = np.zeros((E, 128), np.float32)
        for j in range(4):
            Wp0[:, 32 * j : 32 * j + 16] = W[:, 16 * j : 16 * j + 16]
            Wp1[:, 32 * j : 32 * j + 16] = W[:, 64 + 16 * j : 64 + 16 * j + 16]
        return Wp0, Wp1

    s = np.float32(1.0 / np.sqrt(np.float32(D)))  # 0.25 folded into q weights
    Wq1p0, Wq1p1 = pad4(np.asarray(inputs["Wq_first"], np.float32) * s)
    Wqlp0, Wqlp1 = pad4(np.asarray(inputs["Wq_last"], np.float32) * s)
    Wkp0, Wkp1 = pad4(np.asarray(inputs["Wk"], np.float32))
    return {
        "Wq1p0": Wq1p0,
        "Wq1p1": Wq1p1,
        "Wqlp0": Wqlp0,
        "Wqlp1": Wqlp1,
        "Wkp0": Wkp0,
        "Wkp1": Wkp1,
        "Wv": np.ascontiguousarray(np.asarray(inputs["Wv"], np.float32)),
        "Wc": np.ascontiguousarray(np.asarray(inputs["W_comb"], np.float32)),
        "bc": np.asarray(inputs["b_comb"], np.float32).reshape(E, 1),
    }


def run(inputs, trace=False):
    from concourse.bass_utils import run_bass_kernel_spmd

    if "nc" not in _CACHE:
        _CACHE["nc"] = _build_nc()
    nc = _CACHE["nc"]

    w = _prep_weights(inputs)
    nodes = np.ascontiguousarray(np.asarray(inputs["encoded_nodes"], np.float32))
    q1 = np.ascontiguousarray(np.asarray(inputs["encoded_q1"], np.float32))
    last = np.ascontiguousarray(np.asarray(inputs["encoded_last_node"], np.float32))
    mask = np.ascontiguousarray(np.asarray(inputs["ninf_mask"], np.float32))

    in_maps = []
    for i in range(NCORES):
        sl = slice(i * BL, (i + 1) * BL)
        in_maps.append(
            {
                "nodes": nodes[sl],
                "q1": q1[sl],
                "last": last[sl],
                "mask": mask[sl],
                **w,
            }
        )
    try:
        res = run_bass_kernel_spmd(nc, in_maps, list(range(NCORES)), trace=trace)
    except Exception:
        # The first execution of a freshly compiled NEFF occasionally dies
        # with NRT_EXEC_UNIT_UNRECOVERABLE on this stack; a retry with the
        # cached NEFF has always succeeded.
        res = run_bass_kernel_spmd(nc, in_maps, list(range(NCORES)), trace=trace)
    out = np.concatenate([res.results[i]["probs"] for i in range(NCORES)], axis=0)
    return out, res


def kernel(**inputs) -> np.ndarray:
    out, _ = run(inputs, trace=False)
    return out
